# revision 30
# baseline (speedup 1.0000x reference)
"""Trainium2 Bass kernel for MaskPruningGlobalAttentionChannel.

Reference computation (per batch b, with x = foreground, y = background, m = mask,
all [C, HW] after reshape; bq = bk = bv = 0, gamma scalar):
    corr = (Wq x)(Wk y)^T              [C, C]
    scores = corr m                    [C, HW]
    energy = softmax(scores, axis=-1)
    out = x * m + gamma * (1 - m) * (energy * (Wv x))

Kernel strategy (pure data parallel, one batch per NeuronCore, 8 cores):
  - Gram reassociation: G = x y^T (HW-contraction, fed by host-transposed
    fgT/bgT), then V = G^T Wq^T, corrT = Wk^T-contract V, scores = corrT^T m.
  - Whole score chain in float32r (s1e8m11, 1 PE cycle/row at free-dim >= 256).
    Host pre-rounds inputs to the f32r grid; Wq/Wk are sent as hi+lo f32r
    pairs so their rounding error cancels (residual accumulated in PSUM).
  - Natural-layout fg is NOT transferred: it is rebuilt on-chip from fgT via
    PE transposes (against an identity), saving 4.2 MB of HBM traffic.
  - Blend out = m*x + (1-m)*rr*e*v  (rr = gamma/Z) is 4 elementwise passes:
      p  = m*x          (GpSimd, early, off critical path)    -> fp16
      w' = (m-1)*v      (DVE STT, early)                      -> f32, in-place
      u' = (e*rr)*w'    (DVE STT, tail)  = -(1-m)*rr*e*v      -> fp16 in-place
      out= p - u'       (DVE, tail, fp16 2x mode)             -> bf16 out
  - PSUM is phase-managed: [G-accum + transpose staging] -> released ->
    [Vg/ct + v chunks + scores quarters].
"""

import sys

sys.path.insert(0, "/opt/trn_rl_repo")

import numpy as np

import concourse.bass as bass
import concourse.mybir as mybir
import concourse.tile as tile
from concourse import bacc
from concourse.bass_utils import run_bass_kernel_spmd

B, C, H, W = 8, 256, 64, 64
HW = H * W
NCORES = 8
P = 128
KT = HW // P  # 32 k-tiles over HW for the Gram matmul
F32 = mybir.dt.float32
F32R = mybir.dt.float32r
F16 = mybir.dt.float16
BF16 = mybir.dt.bfloat16
NS = 512  # free-dim chunk for matmuls (one PSUM bank)
NN = HW // NS  # 8
GCH = 4  # k-tiles per G-input DMA chunk
NCH = KT // GCH  # 8
QW = 1024  # scores PSUM quarter width
TC = 2048  # blend chunk width
ACT = mybir.ActivationFunctionType
ALU = mybir.AluOpType

_cache = {}


def _build():
    nc = bacc.Bacc(None)

    fgT = nc.dram_tensor("fgT", [P, KT, C], F32R, kind="ExternalInput")
    bgT = nc.dram_tensor("bgT", [P, KT, C], F32R, kind="ExternalInput")
    msk = nc.dram_tensor("msk", [C, HW], F32R, kind="ExternalInput")
    idn = nc.dram_tensor("idn", [P, P], F32R, kind="ExternalInput")
    # packed weights: [k, slot, c] with slots (WqT_hi, WkT_hi, WvT, WqT_lo, WkT_lo)
    wts = nc.dram_tensor("wts", [P, 2, 5, C], F32R, kind="ExternalInput")
    gam = nc.dram_tensor("gam", [1, 1], F32, kind="ExternalInput")
    out = nc.dram_tensor("out", [C, HW], BF16, kind="ExternalOutput")

    with tile.TileContext(nc) as tc:
        singles = tc.alloc_tile_pool(name="singles", bufs=1)
        gin = tc.alloc_tile_pool(name="gin", bufs=2)
        big = tc.alloc_tile_pool(name="big", bufs=1)
        small = tc.alloc_tile_pool(name="small", bufs=2)
        outp = tc.alloc_tile_pool(name="outp", bufs=2)
        ph1 = tc.alloc_tile_pool(name="ph1", bufs=1, space="PSUM")
        # ph2 allocated after ph1 release

        # ---- persistent SBUF tiles ----
        fg_sb = [big.tile([P, HW], F32R, name=f"fg{m}", tag=f"fg{m}") for m in range(2)]
        msk_sb = [big.tile([P, HW], F32R, name=f"mk{m}", tag=f"mk{m}") for m in range(2)]
        sc_sb = [big.tile([P, HW], F16, name=f"sc{m}", tag=f"sc{m}") for m in range(2)]
        vv_sb = [big.tile([P, HW], F32, name=f"vv{m}", tag=f"vv{m}") for m in range(2)]
        p_sb = [big.tile([P, HW], F16, name=f"pp{m}", tag=f"pp{m}") for m in range(2)]
        sstage = [big.tile([P, HW], F32, name=f"sstage{m}", tag=f"sstage{m}") for m in range(2)]
        wts_sb = singles.tile([P, 2, 5, C], F32R, name="wts", tag="wts")
        idn_sb = singles.tile([P, P], F32R, name="idn", tag="idn")
        gam_sb = singles.tile([P, 1], F32, name="gam", tag="gam")

        def wsl(k, slot):
            return wts_sb[:, k, slot, :]

        # weight DMAs on the ACT HWDGE ring (parallel to Sync's G-input ring)
        nc.scalar.dma_start(idn_sb[:], idn[:, :])
        nc.scalar.dma_start(wts_sb[:], wts[:, :, :, :])
        nc.scalar.dma_start(gam_sb[:], gam.ap().to_broadcast((P, 1)))

        # ---- phase 1: G = sum_hw fgT^T bgT [256, 256]; fg rebuilt via PE
        # transposes of the same fgT chunks ----
        # one full 2KB bank per m-tile: interleaved accumulation groups must
        # not share a PSUM bank (start=True clears the whole bank's
        # has_written bits)
        g_ps = [
            ph1.tile([P, NS], F32, name=f"gps{m}", tag=f"g{m}") for m in range(2)
        ]
        for ch in range(NCH):
            fgt_t = gin.tile([P, GCH, C], F32R, name="fgt", tag="fgt")
            bgt_t = gin.tile([P, GCH, C], F32R, name="bgt", tag="bgt")
            nc.sync.dma_start(fgt_t[:], fgT[:, ch * GCH : (ch + 1) * GCH, :])
            nc.sync.dma_start(bgt_t[:], bgT[:, ch * GCH : (ch + 1) * GCH, :])
            for j in range(GCH):
                t = ch * GCH + j
                for m in range(2):
                    nc.tensor.matmul(
                        g_ps[m][:, 0:C],
                        lhsT=fgt_t[:, j, m * P : (m + 1) * P],
                        rhs=bgt_t[:, j, :],
                        start=(t == 0),
                        stop=(t == KT - 1),
                    )
            for m in range(2):
                tp = ph1.tile([P, GCH * P], F32R, name=f"tp{m}", tag="tp")
                for j in range(GCH):
                    nc.tensor.transpose(
                        tp[:, j * P : (j + 1) * P],
                        fgt_t[:, j, m * P : (m + 1) * P],
                        idn_sb[:],
                    )
                nc.scalar.activation(
                    fg_sb[m][:, ch * GCH * P : (ch + 1) * GCH * P], tp[:], ACT.Copy
                )

        # mask DMAs queue on Sync behind the G inputs, column-halves first
        for c in range(2):
            csl = slice(c * TC, (c + 1) * TC)
            for m in range(2):
                nc.sync.dma_start(msk_sb[m][:, csl], msk[m * P : (m + 1) * P, csl])

        # G hi+lo: gh = f32r(G) via ACT, gl = f32r(G - gh) via DVE
        g_sb = [singles.tile([P, C], F32R, name=f"gsb{m}", tag=f"gsb{m}") for m in range(2)]
        gl_sb = [singles.tile([P, C], F32R, name=f"glsb{m}", tag=f"glsb{m}") for m in range(2)]
        for m in range(2):
            nc.scalar.activation(g_sb[m][:], g_ps[m][:, 0:C], ACT.Copy)
            nc.vector.tensor_tensor(
                out=gl_sb[m][:], in0=g_ps[m][:, 0:C], in1=g_sb[m][:].bitcast(F32),
                op=ALU.subtract,
            )
        ph1.release()

        ph2 = tc.alloc_tile_pool(name="ph2", bufs=1, space="PSUM")

        # ---- phase 2: V[e, c] = sum_f G[f, e] * WqT[f, c] (hi+lo) ----
        vg_ps = ph2.tile([P, 2, C], F32, name="vgps", tag="sm")
        vg_sb = [singles.tile([P, C], F32R, name=f"vgsb{m}", tag=f"vgsb{m}") for m in range(2)]
        vl_sb = [singles.tile([P, C], F32R, name=f"vlsb{m}", tag=f"vlsb{m}") for m in range(2)]
        for me in range(2):
            mms = [(g_sb[kf], kf, s) for kf in range(2) for s in (0, 3)]
            mms += [(gl_sb[kf], kf, 0) for kf in range(2)]
            for i, (gt, kf, s) in enumerate(mms):
                nc.tensor.matmul(
                    vg_ps[:, me, :],
                    lhsT=gt[:, me * P : (me + 1) * P],
                    rhs=wsl(kf, s),
                    start=(i == 0),
                    stop=(i == len(mms) - 1),
                )
            nc.scalar.activation(vg_sb[me][:], vg_ps[:, me, :], ACT.Copy)
            nc.vector.tensor_tensor(
                out=vl_sb[me][:], in0=vg_ps[:, me, :], in1=vg_sb[me][:].bitcast(F32),
                op=ALU.subtract,
            )

        # ---- phase 3: corrT[d, c] = sum_e WkT[e, d] * V[e, c] (hi+lo) ----
        ct_ps = ph2.tile([P, 2, C], F32, name="ctps", tag="sm")
        ct_sb = [singles.tile([P, C], F32R, name=f"ctsb{m}", tag=f"ctsb{m}") for m in range(2)]
        cl_sb = [singles.tile([P, C], F32R, name=f"clsb{m}", tag=f"clsb{m}") for m in range(2)]
        for md in range(2):
            mms = [(ke, s, vg_sb[ke]) for ke in range(2) for s in (1, 4)]
            mms += [(ke, 1, vl_sb[ke]) for ke in range(2)]
            for i, (ke, s, vt) in enumerate(mms):
                nc.tensor.matmul(
                    ct_ps[:, md, :],
                    lhsT=wsl(ke, s)[:, md * P : (md + 1) * P],
                    rhs=vt[:],
                    start=(i == 0),
                    stop=(i == len(mms) - 1),
                )
            nc.scalar.activation(ct_sb[md][:], ct_ps[:, md, :], ACT.Copy)
            nc.vector.tensor_tensor(
                out=cl_sb[md][:], in0=ct_ps[:, md, :], in1=ct_sb[md][:].bitcast(F32),
                op=ALU.subtract,
            )

        # ---- v = WvT-contract-fg, per m-tile, chunked; copies feed vv_sb ----
        def v_phase(mc):
            for n in range(NN):
                sl = slice(n * NS, (n + 1) * NS)
                vp = ph2.tile([P, NS], F32, name="vvps", tag="vp")
                for kc in range(2):
                    nc.tensor.matmul(
                        vp[:],
                        lhsT=wsl(kc, 2)[:, mc * P : (mc + 1) * P],
                        rhs=fg_sb[kc][:, sl],
                        start=(kc == 0),
                        stop=(kc == 1),
                    )
                nc.scalar.activation(vv_sb[mc][:, sl], vp[:], ACT.Copy)

        v_phase(0)
        v_phase(1)

        # early blend passes:  w' = (m-1)*v on DVE (in-place on vv),
        # p = m*x on GpSimd (fp16)
        def wprime(mc, c):
            csl = slice(c * TC, (c + 1) * TC)
            nc.vector.scalar_tensor_tensor(
                out=vv_sb[mc][:, csl], in0=msk_sb[mc][:, csl].bitcast(F32),
                scalar=1.0, in1=vv_sb[mc][:, csl], op0=ALU.subtract, op1=ALU.mult,
            )

        def p_pass(mc, c, half=None):
            csl = slice(c * TC + (0 if half != 1 else TC // 2),
                        c * TC + (TC // 2 if half == 0 else TC))
            eng = nc.gpsimd if half is None or half == 0 else nc.vector
            eng.tensor_mul(
                p_sb[mc][:, csl], msk_sb[mc][:, csl].bitcast(F32),
                fg_sb[mc][:, csl].bitcast(F32),
            )

        # DVE: w' per m-tile as soon as its v copies + mask are in
        wprime(0, 0)
        wprime(0, 1)
        wprime(1, 0)
        wprime(1, 1)
        # GPS queue: three full p chunks + first half of the last; DVE covers
        # the final half so GPS never gates the tail.
        p_pass(0, 0)
        p_pass(0, 1)
        p_pass(1, 0)

        # ---- scores + softmax + tail blend ----
        mxn = [None, None]
        rr = [None, None]

        def scores_phase(mc):
            cmax = small.tile([P, 4], F32, name=f"cmax{mc}", tag=f"cmax{mc}")
            for q in range(4):
                sq = ph2.tile([P, QW], F32, name="scq", tag="scq")
                for h in range(2):
                    n = q * 2 + h
                    sl = slice(n * NS, (n + 1) * NS)
                    mms = [(ct_sb, 0), (ct_sb, 1), (cl_sb, 0), (cl_sb, 1)]
                    for i, (ct, kd) in enumerate(mms):
                        nc.tensor.matmul(
                            sq[:, h * NS : (h + 1) * NS],
                            lhsT=ct[kd][:, mc * P : (mc + 1) * P],
                            rhs=msk_sb[kd][:, sl],
                            start=(i == 0),
                            stop=(i == len(mms) - 1),
                        )
                nc.scalar.activation(
                    sstage[mc][:, q * QW : (q + 1) * QW], sq[:], ACT.Copy
                )
                nc.vector.tensor_reduce(
                    cmax[:, q : q + 1], sq[:], axis=mybir.AxisListType.X, op=ALU.max
                )
            mxn[mc] = small.tile([P, 1], F32, name=f"mxn{mc}", tag=f"mxn{mc}")
            nc.vector.tensor_reduce(
                mxn[mc][:], cmax[:], axis=mybir.AxisListType.X, op=ALU.max, negate=True
            )

        def exp_phase(mc):
            # e = exp(s - max) -> fp16, Z accumulated over the whole row
            zz = small.tile([P, 1], F32, name=f"zz{mc}", tag=f"zz{mc}")
            nc.scalar.activation(
                sc_sb[mc][:], sstage[mc][:], ACT.Exp, bias=mxn[mc][:], accum_out=zz[:]
            )
            rr[mc] = small.tile([P, 1], F32, name=f"rr{mc}", tag=f"rr{mc}")
            nc.vector.reciprocal(rr[mc][:], zz[:])
            nc.vector.tensor_scalar_mul(rr[mc][:], rr[mc][:], gam_sb[:])

        def tail(mc, c):
            csl = slice(c * TC, (c + 1) * TC)
            # u' = (e * rr) * w'  (fp16 in-place on sc)
            nc.vector.scalar_tensor_tensor(
                out=sc_sb[mc][:, csl], in0=sc_sb[mc][:, csl], scalar=rr[mc][:],
                in1=vv_sb[mc][:, csl], op0=ALU.mult, op1=ALU.mult,
            )
            # out = p - u'  (fp16 inputs, bf16 out)
            ob = outp.tile([P, TC], BF16, name="ob", tag="ob")
            nc.vector.tensor_tensor(
                out=ob[:], in0=p_sb[mc][:, csl], in1=sc_sb[mc][:, csl],
                op=ALU.subtract,
            )
            nc.sync.dma_start(out[mc * P : (mc + 1) * P, csl], ob[:])

        scores_phase(0)
        scores_phase(1)
        exp_phase(0)
        tail(0, 0)
        tail(0, 1)
        exp_phase(1)
        tail(1, 0)
        p_pass(1, 1, half=0)  # GPS
        p_pass(1, 1, half=1)  # DVE picks up the last half-chunk
        tail(1, 1)
        ph2.release()
        outp.release()
        small.release()
        big.release()
        gin.release()
        singles.release()

    nc.compile()
    return nc


def _get_nc():
    if "nc" not in _cache:
        _cache["nc"] = _build()
    return _cache["nc"]


def _round_f32r(x):
    # RNE to the fp32r grid: s1e8m11 (drop the low 12 mantissa bits)
    u = np.ascontiguousarray(x, dtype=np.float32).view(np.uint32)
    u = u + 0x7FF + ((u >> 12) & 1)
    u &= np.uint32(0xFFFFF000)
    return u.view(np.float32)


def _prep_inputs(foreground, background, mask, Wq, bq, Wk, bk, Wv, bv, gamma):
    f32 = np.float32
    fg = _round_f32r(np.asarray(foreground, f32).reshape(B, C, HW))
    bg = _round_f32r(np.asarray(background, f32).reshape(B, C, HW))
    mk = _round_f32r(np.asarray(mask, f32).reshape(B, C, HW))
    wqt = np.asarray(Wq, f32).T.astype(np.float64)
    wkt = np.asarray(Wk, f32).T.astype(np.float64)
    wqh = _round_f32r(wqt)
    wql = _round_f32r(wqt - wqh)
    wkh = _round_f32r(wkt)
    wkl = _round_f32r(wkt - wkh)
    wvt = _round_f32r(np.asarray(Wv, f32).T)
    gm = np.asarray(gamma, f32).reshape(1, 1)

    # packed weights [P, 2, 5, C]
    wts = np.empty((P, 2, 5, C), f32)
    for k in range(2):
        rows = slice(k * P, (k + 1) * P)
        wts[:, k, 0] = wqh[rows]
        wts[:, k, 1] = wkh[rows]
        wts[:, k, 2] = wvt[rows]
        wts[:, k, 3] = wql[rows]
        wts[:, k, 4] = wkl[rows]
    idn = np.eye(P, dtype=f32)

    def blocked_T(x):  # x: [C, HW] -> [P, KT, C]
        return np.ascontiguousarray(x.T.reshape(KT, P, C).transpose(1, 0, 2))

    in_maps = []
    for b in range(B):
        in_maps.append(
            {
                "fgT": blocked_T(fg[b]),
                "bgT": blocked_T(bg[b]),
                "msk": np.ascontiguousarray(mk[b]),
                "idn": idn,
                "wts": wts,
                "gam": gm,
            }
        )
    return in_maps


def run(inputs, trace=False, tmpdir=None):
    nc = _get_nc()
    in_maps = _prep_inputs(**inputs)
    res = run_bass_kernel_spmd(
        nc, in_maps, core_ids=list(range(NCORES)), trace=trace, tmpdir=tmpdir
    )
    outs = np.stack(
        [np.asarray(res.results[i]["out"]) for i in range(NCORES)], axis=0
    )
    return outs.reshape(B, C, H, W).astype(np.float32), res


def kernel(**inputs):
    out, _ = run(inputs, trace=False)
    return out


# revision 39
# speedup vs baseline: 1.1706x; 1.1706x over previous
"""Trainium2 Bass kernel for MaskPruningGlobalAttentionChannel.

Reference computation (per batch b, with x = foreground, y = background, m = mask,
all [C, HW] after reshape; bq = bk = bv = 0, gamma scalar):
    corr = (Wq x)(Wk y)^T              [C, C]
    scores = corr m                    [C, HW]
    energy = softmax(scores, axis=-1)
    out = x * m + gamma * (1 - m) * (energy * (Wv x))

Kernel strategy (pure data parallel, one batch per NeuronCore, 8 cores):
  - Gram reassociation: G = x y^T (HW-contraction, fed by host-transposed
    fgT/bgT), then V = G^T Wq^T, corrT = Wk^T-contract V, scores = corrT^T m.
  - Whole score chain in float32r (s1e8m11, 1 PE cycle/row at free-dim >= 256).
    Host pre-rounds inputs to the f32r grid; Wq/Wk are sent as hi+lo f32r
    pairs so their rounding error cancels (residual accumulated in PSUM).
  - Natural-layout fg is NOT transferred: it is rebuilt on-chip from fgT via
    PE transposes (against an identity), saving 4.2 MB of HBM traffic.
  - Blend out = m*x + (1-m)*rr*e*v  (rr = gamma/Z) is 4 elementwise passes:
      p  = m*x          (GpSimd, early, off critical path)    -> fp16
      w' = (m-1)*v      (DVE STT, early)                      -> f32, in-place
      u' = (e*rr)*w'    (DVE STT, tail)  = -(1-m)*rr*e*v      -> fp16 in-place
      out= p - u'       (DVE, tail, fp16 2x mode)             -> bf16 out
  - PSUM is phase-managed: [G-accum + transpose staging] -> released ->
    [Vg/ct + v chunks + scores quarters].
"""

import sys

sys.path.insert(0, "/opt/trn_rl_repo")

import numpy as np

import concourse.bass as bass
import concourse.mybir as mybir
import concourse.tile as tile
from concourse import bacc
from concourse.bass_utils import run_bass_kernel_spmd

B, C, H, W = 8, 256, 64, 64
HW = H * W
NCORES = 8
P = 128
KT = HW // P  # 32 k-tiles over HW for the Gram matmul
F32 = mybir.dt.float32
F32R = mybir.dt.float32r
F16 = mybir.dt.float16
BF16 = mybir.dt.bfloat16
NS = 512  # free-dim chunk for matmuls (one PSUM bank)
NN = HW // NS  # 8
GCH = 8  # k-tiles per G-input DMA chunk
NCH = KT // GCH  # 8
QW = 1024  # scores PSUM quarter width
TC = 2048  # blend chunk width
ACT = mybir.ActivationFunctionType
ALU = mybir.AluOpType

_cache = {}


def _build():
    nc = bacc.Bacc(None)

    fgT = nc.dram_tensor("fgT", [P, KT, C], F32R, kind="ExternalInput")
    bgT = nc.dram_tensor("bgT", [P, KT, C], F32R, kind="ExternalInput")
    msk = nc.dram_tensor("msk", [C, HW], F32R, kind="ExternalInput")
    idn = nc.dram_tensor("idn", [P, P], F32R, kind="ExternalInput")
    # packed weights: [k, slot, c] with slots (WqT_hi, WkT_hi, WvT, WqT_lo, WkT_lo)
    wts = nc.dram_tensor("wts", [P, 2, 5, C], F32R, kind="ExternalInput")
    gam = nc.dram_tensor("gam", [1, 1], F32, kind="ExternalInput")
    out = nc.dram_tensor("out", [C, HW], BF16, kind="ExternalOutput")

    with tile.TileContext(nc) as tc:
        singles = tc.alloc_tile_pool(name="singles", bufs=1)
        gin = tc.alloc_tile_pool(name="gin", bufs=2)
        big = tc.alloc_tile_pool(name="big", bufs=1)
        small = tc.alloc_tile_pool(name="small", bufs=2)
        outp = tc.alloc_tile_pool(name="outp", bufs=2)
        ph1 = tc.alloc_tile_pool(name="ph1", bufs=1, space="PSUM")
        # ph2 allocated after ph1 release

        # ---- persistent SBUF tiles ----
        fg_sb = [big.tile([P, HW], F32R, name=f"fg{m}", tag=f"fg{m}") for m in range(2)]
        msk_sb = [big.tile([P, HW], F32R, name=f"mk{m}", tag=f"mk{m}") for m in range(2)]
        sc_sb = [big.tile([P, HW], F16, name=f"sc{m}", tag=f"sc{m}") for m in range(2)]
        vv_sb = [big.tile([P, HW], F16, name=f"vv{m}", tag=f"vv{m}") for m in range(2)]
        p_sb = [big.tile([P, HW], F16, name=f"pp{m}", tag=f"pp{m}") for m in range(2)]
        sstage = [big.tile([P, HW], F32, name=f"sstage{m}", tag=f"sstage{m}") for m in range(2)]
        wts_sb = singles.tile([P, 2, 5, C], F32R, name="wts", tag="wts")
        idn_sb = singles.tile([P, P], F32R, name="idn", tag="idn")
        gam_sb = singles.tile([P, 1], F32, name="gam", tag="gam")

        def wsl(k, slot):
            return wts_sb[:, k, slot, :]

        # identity first on the ACT HWDGE ring (transposes need it early);
        # the big wts transfer is emitted mid-G-loop so bgt chunks 0-1 lead
        nc.scalar.dma_start(idn_sb[:], idn[:, :])

        # ---- phase 1: G = sum_hw fgT^T bgT [256, 256]; fg rebuilt via PE
        # transposes of the same fgT chunks ----
        # one full 2KB bank per m-tile: interleaved accumulation groups must
        # not share a PSUM bank (start=True clears the whole bank's
        # has_written bits)
        g_ps = [
            ph1.tile([P, NS], F32, name=f"gps{m}", tag=f"g{m}") for m in range(2)
        ]
        for ch in range(NCH):
            fgt_t = gin.tile([P, GCH, C], F32R, name="fgt", tag="fgt")
            bgt_t = gin.tile([P, GCH, C], F32R, name="bgt", tag="bgt")
            # one 1 MB transfer per tensor per chunk, split across the two
            # HWDGE rings (Sync + Scalar) so their fixed costs overlap
            nc.sync.dma_start(fgt_t[:], fgT[:, ch * GCH : (ch + 1) * GCH, :])
            nc.scalar.dma_start(bgt_t[:], bgT[:, ch * GCH : (ch + 1) * GCH, :])
            if ch == 1:
                nc.scalar.dma_start(wts_sb[:], wts[:, :, :, :])
                nc.scalar.dma_start(gam_sb[:], gam.ap().to_broadcast((P, 1)))
            for j in range(GCH):
                t = ch * GCH + j
                for m in range(2):
                    nc.tensor.matmul(
                        g_ps[m][:, 0:C],
                        lhsT=fgt_t[:, j, m * P : (m + 1) * P],
                        rhs=bgt_t[:, j, :],
                        start=(t == 0),
                        stop=(t == KT - 1),
                    )
            for m in range(2):
                tp = ph1.tile([P, GCH * P], F32R, name=f"tp{m}", tag="tp", bufs=2)
                for j in range(GCH):
                    nc.tensor.transpose(
                        tp[:, j * P : (j + 1) * P],
                        fgt_t[:, j, m * P : (m + 1) * P],
                        idn_sb[:],
                    )
                nc.scalar.activation(
                    fg_sb[m][:, ch * GCH * P : (ch + 1) * GCH * P], tp[:], ACT.Copy
                )

        # mask DMAs queue behind the G inputs, column-halves first, one ring
        # per m-tile
        for c in range(2):
            csl = slice(c * TC, (c + 1) * TC)
            nc.sync.dma_start(msk_sb[0][:, csl], msk[0:P, csl])
            nc.scalar.dma_start(msk_sb[1][:, csl], msk[P : 2 * P, csl])

        # G hi+lo: gh = f32r(G) via ACT, gl = f32r(G - gh) via DVE
        g_sb = [singles.tile([P, C], F32R, name=f"gsb{m}", tag=f"gsb{m}") for m in range(2)]
        gl_sb = [singles.tile([P, C], F32R, name=f"glsb{m}", tag=f"glsb{m}") for m in range(2)]
        for m in range(2):
            nc.scalar.activation(g_sb[m][:], g_ps[m][:, 0:C], ACT.Copy)
            nc.vector.tensor_tensor(
                out=gl_sb[m][:], in0=g_ps[m][:, 0:C], in1=g_sb[m][:].bitcast(F32),
                op=ALU.subtract,
            )
        ph1.release()

        ph2 = tc.alloc_tile_pool(name="ph2", bufs=1, space="PSUM")

        # ---- phase 2: V[e, c] = sum_f G[f, e] * WqT[f, c] (hi+lo) ----
        vg_ps = ph2.tile([P, 2, C], F32, name="vgps", tag="sm")
        vg_sb = [singles.tile([P, C], F32R, name=f"vgsb{m}", tag=f"vgsb{m}") for m in range(2)]
        vl_sb = [singles.tile([P, C], F32R, name=f"vlsb{m}", tag=f"vlsb{m}") for m in range(2)]
        for me in range(2):
            mms = [(g_sb[kf], kf, s) for kf in range(2) for s in (0, 3)]
            mms += [(gl_sb[kf], kf, 0) for kf in range(2)]
            for i, (gt, kf, s) in enumerate(mms):
                nc.tensor.matmul(
                    vg_ps[:, me, :],
                    lhsT=gt[:, me * P : (me + 1) * P],
                    rhs=wsl(kf, s),
                    start=(i == 0),
                    stop=(i == len(mms) - 1),
                )
            nc.scalar.activation(vg_sb[me][:], vg_ps[:, me, :], ACT.Copy)
            nc.vector.tensor_tensor(
                out=vl_sb[me][:], in0=vg_ps[:, me, :], in1=vg_sb[me][:].bitcast(F32),
                op=ALU.subtract,
            )

        # ---- phase 3: corrT[d, c] = sum_e WkT[e, d] * V[e, c] (hi+lo) ----
        ct_ps = ph2.tile([P, 2, C], F32, name="ctps", tag="sm")
        ct_sb = [singles.tile([P, C], F32R, name=f"ctsb{m}", tag=f"ctsb{m}") for m in range(2)]
        cl_sb = [singles.tile([P, C], F32R, name=f"clsb{m}", tag=f"clsb{m}") for m in range(2)]
        for md in range(2):
            mms = [(ke, s, vg_sb[ke]) for ke in range(2) for s in (1, 4)]
            mms += [(ke, 1, vl_sb[ke]) for ke in range(2)]
            for i, (ke, s, vt) in enumerate(mms):
                nc.tensor.matmul(
                    ct_ps[:, md, :],
                    lhsT=wsl(ke, s)[:, md * P : (md + 1) * P],
                    rhs=vt[:],
                    start=(i == 0),
                    stop=(i == len(mms) - 1),
                )
            nc.scalar.activation(ct_sb[md][:], ct_ps[:, md, :], ACT.Copy)
            nc.vector.tensor_tensor(
                out=cl_sb[md][:], in0=ct_ps[:, md, :], in1=ct_sb[md][:].bitcast(F32),
                op=ALU.subtract,
            )

        # ---- v = WvT-contract-fg, per m-tile, chunked; copies feed vv_sb ----
        def v_phase(mc):
            for n in range(NN):
                sl = slice(n * NS, (n + 1) * NS)
                vp = ph2.tile([P, NS], F32, name="vvps", tag="vp", bufs=2)
                for kc in range(2):
                    nc.tensor.matmul(
                        vp[:],
                        lhsT=wsl(kc, 2)[:, mc * P : (mc + 1) * P],
                        rhs=fg_sb[kc][:, sl],
                        start=(kc == 0),
                        stop=(kc == 1),
                    )
                nc.scalar.activation(vv_sb[mc][:, sl], vp[:], ACT.Copy)

        v_phase(0)
        v_phase(1)

        # early blend passes:  w' = (m-1)*v on DVE (in-place on vv),
        # p = m*x on GpSimd (fp16)
        def wprime(mc, c):
            csl = slice(c * TC, (c + 1) * TC)
            nc.vector.scalar_tensor_tensor(
                out=vv_sb[mc][:, csl], in0=msk_sb[mc][:, csl].bitcast(F32),
                scalar=1.0, in1=vv_sb[mc][:, csl], op0=ALU.subtract, op1=ALU.mult,
            )

        def p_pass(mc, c, half=None):
            csl = slice(c * TC + (0 if half != 1 else TC // 2),
                        c * TC + (TC // 2 if half == 0 else TC))
            eng = nc.gpsimd if half is None or half == 0 else nc.vector
            eng.tensor_mul(
                p_sb[mc][:, csl], msk_sb[mc][:, csl].bitcast(F32),
                fg_sb[mc][:, csl].bitcast(F32),
            )

        # DVE: w' per m-tile as soon as its v copies + mask are in
        wprime(0, 0)
        wprime(0, 1)
        wprime(1, 0)
        wprime(1, 1)
        # GPS queue: three full p chunks + first half of the last; DVE covers
        # the final half so GPS never gates the tail.
        p_pass(0, 0)
        p_pass(0, 1)
        p_pass(1, 0)

        # ---- scores + softmax + tail blend ----
        mxn = [None, None]
        rr = [None, None]

        def scores_phase(mc):
            cmax = small.tile([P, 4], F32, name=f"cmax{mc}", tag=f"cmax{mc}")
            for q in range(4):
                sq = ph2.tile([P, QW], F32, name="scq", tag="scq", bufs=2)
                for h in range(2):
                    n = q * 2 + h
                    sl = slice(n * NS, (n + 1) * NS)
                    mms = [(ct_sb, 0), (ct_sb, 1), (cl_sb, 0), (cl_sb, 1)]
                    for i, (ct, kd) in enumerate(mms):
                        nc.tensor.matmul(
                            sq[:, h * NS : (h + 1) * NS],
                            lhsT=ct[kd][:, mc * P : (mc + 1) * P],
                            rhs=msk_sb[kd][:, sl],
                            start=(i == 0),
                            stop=(i == len(mms) - 1),
                        )
                nc.scalar.activation(
                    sstage[mc][:, q * QW : (q + 1) * QW], sq[:], ACT.Copy
                )
                nc.vector.tensor_reduce(
                    cmax[:, q : q + 1], sq[:], axis=mybir.AxisListType.X, op=ALU.max
                )
            mxn[mc] = small.tile([P, 1], F32, name=f"mxn{mc}", tag=f"mxn{mc}")
            nc.vector.tensor_reduce(
                mxn[mc][:], cmax[:], axis=mybir.AxisListType.X, op=ALU.max, negate=True
            )

        def exp_phase(mc):
            # e = exp(s - max) -> fp16, Z accumulated over the whole row
            zz = small.tile([P, 1], F32, name=f"zz{mc}", tag=f"zz{mc}")
            nc.scalar.activation(
                sc_sb[mc][:], sstage[mc][:], ACT.Exp, bias=mxn[mc][:], accum_out=zz[:]
            )
            rr[mc] = small.tile([P, 1], F32, name=f"rr{mc}", tag=f"rr{mc}")
            nc.vector.reciprocal(rr[mc][:], zz[:])
            nc.vector.tensor_scalar_mul(rr[mc][:], rr[mc][:], gam_sb[:])

        def tail(mc, c):
            csl = slice(c * TC, (c + 1) * TC)
            # u' = (e * rr) * w'  (fp16 in-place on sc)
            nc.vector.scalar_tensor_tensor(
                out=sc_sb[mc][:, csl], in0=sc_sb[mc][:, csl], scalar=rr[mc][:],
                in1=vv_sb[mc][:, csl], op0=ALU.mult, op1=ALU.mult,
            )
            # out = p - u'  (fp16 inputs, bf16 out)
            ob = outp.tile([P, TC], BF16, name="ob", tag="ob")
            nc.vector.tensor_tensor(
                out=ob[:], in0=p_sb[mc][:, csl], in1=sc_sb[mc][:, csl],
                op=ALU.subtract,
            )
            nc.sync.dma_start(out[mc * P : (mc + 1) * P, csl], ob[:])

        scores_phase(0)
        scores_phase(1)
        exp_phase(0)
        tail(0, 0)
        tail(0, 1)
        exp_phase(1)
        tail(1, 0)
        p_pass(1, 1, half=0)  # GPS
        p_pass(1, 1, half=1)  # DVE picks up the last half-chunk
        tail(1, 1)
        ph2.release()
        outp.release()
        small.release()
        big.release()
        gin.release()
        singles.release()

    nc.compile()
    return nc


def _get_nc():
    if "nc" not in _cache:
        _cache["nc"] = _build()
    return _cache["nc"]


def _round_f32r(x):
    # RNE to the fp32r grid: s1e8m11 (drop the low 12 mantissa bits)
    u = np.ascontiguousarray(x, dtype=np.float32).view(np.uint32)
    u = u + 0x7FF + ((u >> 12) & 1)
    u &= np.uint32(0xFFFFF000)
    return u.view(np.float32)


def _prep_inputs(foreground, background, mask, Wq, bq, Wk, bk, Wv, bv, gamma):
    f32 = np.float32
    fg = _round_f32r(np.asarray(foreground, f32).reshape(B, C, HW))
    bg = _round_f32r(np.asarray(background, f32).reshape(B, C, HW))
    mk = _round_f32r(np.asarray(mask, f32).reshape(B, C, HW))
    wqt = np.asarray(Wq, f32).T.astype(np.float64)
    wkt = np.asarray(Wk, f32).T.astype(np.float64)
    wqh = _round_f32r(wqt)
    wql = _round_f32r(wqt - wqh)
    wkh = _round_f32r(wkt)
    wkl = _round_f32r(wkt - wkh)
    wvt = _round_f32r(np.asarray(Wv, f32).T)
    gm = np.asarray(gamma, f32).reshape(1, 1)

    # packed weights [P, 2, 5, C]
    wts = np.empty((P, 2, 5, C), f32)
    for k in range(2):
        rows = slice(k * P, (k + 1) * P)
        wts[:, k, 0] = wqh[rows]
        wts[:, k, 1] = wkh[rows]
        wts[:, k, 2] = wvt[rows]
        wts[:, k, 3] = wql[rows]
        wts[:, k, 4] = wkl[rows]
    idn = np.eye(P, dtype=f32)

    def blocked_T(x):  # x: [C, HW] -> [P, KT, C]
        return np.ascontiguousarray(x.T.reshape(KT, P, C).transpose(1, 0, 2))

    in_maps = []
    for b in range(B):
        in_maps.append(
            {
                "fgT": blocked_T(fg[b]),
                "bgT": blocked_T(bg[b]),
                "msk": np.ascontiguousarray(mk[b]),
                "idn": idn,
                "wts": wts,
                "gam": gm,
            }
        )
    return in_maps


def run(inputs, trace=False, tmpdir=None):
    nc = _get_nc()
    in_maps = _prep_inputs(**inputs)
    res = run_bass_kernel_spmd(
        nc, in_maps, core_ids=list(range(NCORES)), trace=trace, tmpdir=tmpdir
    )
    outs = np.stack(
        [np.asarray(res.results[i]["out"]) for i in range(NCORES)], axis=0
    )
    return outs.reshape(B, C, H, W).astype(np.float32), res


def kernel(**inputs):
    out, _ = run(inputs, trace=False)
    return out
